# revision 5
# baseline (speedup 1.0000x reference)
"""2-layer GraphConv GNN on 8 trn2 NeuronCores (Bass/Tile) — v6.

(promoted to kernel.py)

Instruction-count-minimal design (~330 instrs/iter vs baseline ~700):
  - Edges sharded by dst core; slots streamed per (src-range 4 x dst-half 2),
    chunked at CH idxs. Per chunk: 1 transposed dma_gather (feature-major
    msgs), 1 DVE strided copy into a zero-interleaved buffer, 1 gpsimd SBUF
    scatter_add (d=2, j=1 lanes permanently zero).
  - scatter_add loses adds on duplicate idx closer than 8 slots; host orders
    each chunk occurrence-major with every occurrence section padded to >=16,
    which provably spaces equal-dst slots >= 16 apart.
  - Transforms feature-major, 512-wide psum batches: h = relu(W1@aggT +
    W0@xsT + b) via scalar.activation per-partition bias.
  - Feature-major -> node-major conversion (for gather tables / output) via
    SBUF-source transposed dma_gather token trick (2 instrs per conversion).
  - Tables padded to 12544 rows/shard so src row = (src//12500)*12544 +
    src%12500; the same gather idx stream serves both layers.
"""

import os
import numpy as np
import ml_dtypes
from contextlib import ExitStack

N = 100000
F = 128
O = 64
NC = 8
SHARD = 12500
SH2 = 12544              # padded shard rows (98*128)
N2 = NC * SH2            # 100352
NR = 4
RSRC = N2 // NR          # 25088 (int16 gather idx)
NH = 2
HB = 6272                # dst-half size (49*128)
NEH = 6288               # agg num_elems: HB + 16 dump rows
DUMP = HB                # dump dst row
CH = 8704                # slots per chunk (mult of 128)
CR = CH - 256            # raw edges per chunk before section padding
MINSP = 48               # duplicate spacing target

bf16 = ml_dtypes.bfloat16

_LOOP = int(os.environ.get("GNN_LOOP", "1"))

# set by prepare_in_maps(): NCH[r][h] chunks per stream (shared across cores),
# SOFF[r][h] slot offset of stream, GW total slots.
_PROF = {}


def input_decls():
    GW = _PROF["GW"]
    return [
        ("xs", [SH2, F], "bfloat16"),
        ("gidx", [16, GW // 16], "int16"),
        ("sidx", [16, GW // 16], "int16"),
        ("aidx", [16, SH2 // 16], "int16"),
        ("wr1T", [F, F], "bfloat16"),
        ("wo1T", [F, F], "bfloat16"),
        ("wr2T", [F, O], "bfloat16"),
        ("wo2T", [F, O], "bfloat16"),
        ("b1c", [128, 1], "float32"),
        ("b2c", [128, 1], "float32"),
    ]


def _build_program():
    import concourse.bass as bass
    import concourse.tile as tile
    from concourse import bacc, mybir

    NCH = _PROF["NCH"]
    SOFF = _PROF["SOFF"]
    GW = _PROF["GW"]

    nc = bacc.Bacc(None, target_bir_lowering=False, num_swdge_queues=4)
    dt = mybir.dt

    xs_in = nc.dram_tensor("xs", [SH2, F], dt.bfloat16, kind="ExternalInput")
    gidx_in = nc.dram_tensor("gidx", [16, GW // 16], dt.int16, kind="ExternalInput")
    sidx_in = nc.dram_tensor("sidx", [16, GW // 16], dt.int16, kind="ExternalInput")
    aidx_in = nc.dram_tensor("aidx", [16, SH2 // 16], dt.int16, kind="ExternalInput")
    wr1T_in = nc.dram_tensor("wr1T", [F, F], dt.bfloat16, kind="ExternalInput")
    wo1T_in = nc.dram_tensor("wo1T", [F, F], dt.bfloat16, kind="ExternalInput")
    wr2T_in = nc.dram_tensor("wr2T", [F, O], dt.bfloat16, kind="ExternalInput")
    wo2T_in = nc.dram_tensor("wo2T", [F, O], dt.bfloat16, kind="ExternalInput")
    b1c_in = nc.dram_tensor("b1c", [128, 1], dt.float32, kind="ExternalInput")
    b2c_in = nc.dram_tensor("b2c", [128, 1], dt.float32, kind="ExternalInput")
    out_t = nc.dram_tensor("out", [SH2, O], dt.bfloat16, kind="ExternalOutput")

    xs_int = nc.dram_tensor("xs_int", [SH2, F], dt.bfloat16)
    xfull = nc.dram_tensor("xfull", [N2, F], dt.bfloat16, addr_space="Shared")
    h_shard = nc.dram_tensor("h_shard", [SH2, F], dt.bfloat16)
    h_full = nc.dram_tensor("h_full", [N2, F], dt.bfloat16, addr_space="Shared")

    with tile.TileContext(nc) as tc, ExitStack() as ctx:
        const_p = ctx.enter_context(tc.tile_pool(name="const", bufs=1))
        resid_p = ctx.enter_context(tc.tile_pool(name="resid", bufs=1))
        msgs_p = ctx.enter_context(tc.tile_pool(name="msgs", bufs=1))
        agg_p = ctx.enter_context(tc.tile_pool(name="aggp", bufs=1))
        ps_p = ctx.enter_context(tc.tile_pool(name="psp", bufs=2, space="PSUM"))

        # ---- consts ----
        c_wr1T = const_p.tile([F, F], dt.bfloat16)
        nc.sync.dma_start(c_wr1T[:], wr1T_in[:])
        c_wo1T = const_p.tile([F, F], dt.bfloat16)
        nc.sync.dma_start(c_wo1T[:], wo1T_in[:])
        c_wr2T = const_p.tile([F, O], dt.bfloat16)
        nc.sync.dma_start(c_wr2T[:], wr2T_in[:])
        c_wo2T = const_p.tile([F, O], dt.bfloat16)
        nc.sync.dma_start(c_wo2T[:], wo2T_in[:])
        c_b1c = const_p.tile([128, 1], dt.float32)
        nc.sync.dma_start(c_b1c[:], b1c_in[:])
        c_b2c = const_p.tile([128, 1], dt.float32)
        nc.sync.dma_start(c_b2c[:], b2c_in[:])

        # ---- residents ----
        gidx_r = resid_p.tile([128, GW // 16], dt.int16)
        sidx_r = resid_p.tile([128, GW // 16], dt.int16)
        aidx_r = resid_p.tile([128, SH2 // 16], dt.int16)
        for k in range(8):
            nc.sync.dma_start(gidx_r[16 * k : 16 * (k + 1), :], gidx_in[:])
            nc.sync.dma_start(sidx_r[16 * k : 16 * (k + 1), :], sidx_in[:])
            nc.sync.dma_start(aidx_r[16 * k : 16 * (k + 1), :], aidx_in[:])
        xsT = resid_p.tile([F, SH2], dt.bfloat16)
        hT = resid_p.tile([F, SH2], dt.bfloat16)
        outT = resid_p.tile([64, HB + 128], dt.bfloat16)
        agg = agg_p.tile([128, NEH * 2], dt.bfloat16)
        aggTc = agg_p.tile([128, HB], dt.bfloat16)
        mT = msgs_p.tile([128, CH], dt.bfloat16)
        mz = msgs_p.tile([128, CH * 2], dt.bfloat16)
        nc.vector.memset(mz[:], 0.0)  # j=1 lanes stay zero forever
        stg = mT  # conversions stage through mT (idle between agg phases)

        aggv = agg[:].rearrange("p (n d) -> p n d", d=2)
        mzv = mz[:].rearrange("p (n d) -> p n d", d=2)

        def aggregate(h, table):
            """Zero agg, then gather+scatter all 4 range streams of half h."""
            nc.vector.memset(agg[:], 0.0)
            for r in range(NR):
                for c in range(NCH[r][h]):
                    base = SOFF[r][h] + c * CH
                    nc.gpsimd.dma_gather(
                        mT[:].rearrange("p (c e) -> p c e", c=1),
                        table[r * RSRC : (r + 1) * RSRC, :],
                        gidx_r[:, base // 16 : base // 16 + CH // 16],
                        CH,
                        CH,
                        F,
                        transpose=True,
                        single_packet=False,
                        queue_num=0,
                    )
                    nc.vector.tensor_copy(out=mzv[:, :, 0], in_=mT[:])
                    nc.gpsimd.scatter_add(
                        aggv[:],
                        sidx_r[:, base // 16 : base // 16 + CH // 16],
                        mzv[:],
                        128,
                        NEH,
                        2,
                        CH,
                    )
            nc.vector.tensor_copy(out=aggTc[:], in_=aggv[:, :HB, 0])

        def convert_tokens(src_tile, n_tok, tpr, aoff, dst_dram, fw):
            """Feature-major SBUF src -> node-major DRAM rows via SBUF-source
            transposed gather (token v = b*tpr + f) + one dense DMA."""
            nc.gpsimd.dma_gather(
                stg[:, :n_tok].rearrange("p (c e) -> p c e", c=1),
                src_tile,
                aidx_r[:, aoff // 16 : aoff // 16 + n_tok // 16],
                n_tok,
                n_tok,
                F,
                transpose=True,
                single_packet=False,
                queue_num=0,
                sbuf_tokens_per_rank=tpr,
                sbuf_free_dim_per_rank=256,
                sbuf_free_dim_pad_per_rank=0,
                sbuf_byte_offset=0,
            )
            nc.sync.dma_start(
                dst_dram.rearrange("(b p) f -> p b f", p=128),
                stg[:, :n_tok].rearrange("p (b f) -> p b f", f=F),
            )

        NB1 = (HB + 511) // 512  # 13 (12x512 + 128)

        for _it in range(_LOOP):
            nc.sync.dma_start(xs_int[:], xs_in[:])
            nc.gpsimd.collective_compute(
                "AllGather",
                mybir.AluOpType.bypass,
                replica_groups=[list(range(NC))],
                ins=[xs_int[:]],
                outs=[xfull[:]],
            )
            # xsT: feature-major local x via transposed gather from DRAM
            nc.gpsimd.dma_gather(
                xsT[:].rearrange("p (c e) -> p c e", c=1),
                xs_int[:],
                aidx_r[:],
                SH2,
                SH2,
                F,
                transpose=True,
                single_packet=False,
                queue_num=0,
            )
            # ================= layer 1 =================
            for h in range(NH):
                aggregate(h, xfull)
                for b in range(NB1):
                    lo = b * 512
                    w = min(512, HB - lo)
                    ps = ps_p.tile([128, 512], dt.float32, tag="ps", space="PSUM")
                    nc.tensor.matmul(
                        ps[:, :w], lhsT=c_wr1T[:], rhs=aggTc[:, lo : lo + w],
                        start=True, stop=False,
                    )
                    nc.tensor.matmul(
                        ps[:, :w], lhsT=c_wo1T[:],
                        rhs=xsT[:, h * HB + lo : h * HB + lo + w],
                        start=False, stop=True,
                    )
                    nc.scalar.activation(
                        out=hT[:, h * HB + lo : h * HB + lo + w],
                        in_=ps[:, :w],
                        func=mybir.ActivationFunctionType.Relu,
                        bias=c_b1c[:],
                    )
            # h table: feature-major -> node-major DRAM, 4 block chunks
            for (b0, b1) in ((0, 25), (25, 49), (49, 74), (74, 98)):
                convert_tokens(
                    hT[:],
                    (b1 - b0) * 128,
                    128,
                    b0 * 128,
                    h_shard[b0 * 128 : b1 * 128, :],
                    F,
                )
            nc.gpsimd.collective_compute(
                "AllGather",
                mybir.AluOpType.bypass,
                replica_groups=[list(range(NC))],
                ins=[h_shard[:]],
                outs=[h_full[:]],
            )
            # ================= layer 2 =================
            for h in range(NH):
                aggregate(h, h_full)
                for b in range(NB1):
                    lo = b * 512
                    w = min(512, HB - lo)
                    ps = ps_p.tile([128, 512], dt.float32, tag="ps", space="PSUM")
                    nc.tensor.matmul(
                        ps[0:64, :w], lhsT=c_wr2T[:], rhs=aggTc[:, lo : lo + w],
                        start=True, stop=False,
                    )
                    nc.tensor.matmul(
                        ps[0:64, :w], lhsT=c_wo2T[:],
                        rhs=hT[:, h * HB + lo : h * HB + lo + w],
                        start=False, stop=True,
                    )
                    nc.vector.tensor_scalar(
                        out=outT[0:64, lo : lo + w],
                        in0=ps[0:64, :w],
                        scalar1=c_b2c[0:64, :],
                        scalar2=None,
                        op0=mybir.AluOpType.add,
                    )
                # out conversion: tokens (f<64, b<49): v = b*64+f, 256B stripes
                nc.gpsimd.dma_gather(
                    stg[:, :3200].rearrange("p (c e) -> p c e", c=1),
                    outT[:],
                    aidx_r[:, :200],
                    3200,
                    3200,
                    F,
                    transpose=True,
                    single_packet=False,
                    queue_num=0,
                    sbuf_tokens_per_rank=64,
                    sbuf_free_dim_per_rank=256,
                    sbuf_free_dim_pad_per_rank=0,
                    sbuf_byte_offset=0,
                )
                nc.sync.dma_start(
                    out_t[h * HB : (h + 1) * HB, :].rearrange(
                        "(b p) f -> p b f", p=128
                    ),
                    stg[:, : HB // 2].rearrange("p (b f) -> p b f", f=O),
                )

    nc.finalize()
    return nc


_CACHED = {}


def _wrap16(flat):
    return np.ascontiguousarray(flat.reshape(-1, 16).T)


def _space_chunk(darr):
    """Order a chunk's dst values occurrence-major with every occurrence
    section padded to >= MINSP slots; guarantees equal-dst spacing >= MINSP.
    Returns (perm indices into darr, pad positions mask) as final slot list:
    list of (edge_idx or -1 for dump)."""
    m = len(darr)
    if m == 0:
        return np.full(0, -1, np.int64)
    o = np.argsort(darr, kind="stable")
    ds = darr[o]
    grp_start = np.r_[0, np.flatnonzero(np.diff(ds)) + 1]
    occ = np.arange(m) - np.repeat(grp_start, np.diff(np.r_[grp_start, m]))
    occ_of = np.empty(m, np.int64)
    occ_of[o] = occ
    fin = np.lexsort((darr, occ_of))  # primary occ, secondary dst
    nk = np.bincount(occ_of)
    slots = []
    pos = 0
    for k, n_k in enumerate(nk):
        slots.append(fin[pos : pos + n_k])
        pos += n_k
        # spacing bound for occ k -> k+1 is n_{k+1} + pad_k; pad so it >= MINSP
        if k + 1 < len(nk) and nk[k + 1] < MINSP:
            slots.append(np.full(MINSP - nk[k + 1], -1, np.int64))
    out = np.concatenate(slots)
    # verify: equal-dst slot positions differ by >= 8 (hw hazard window)
    val = out >= 0
    posi = np.flatnonzero(val)
    dv = darr[out[val]]
    o2 = np.lexsort((posi, dv))
    ds2 = dv[o2]
    ps2 = posi[o2]
    same = ds2[1:] == ds2[:-1]
    if same.any():
        mind = (ps2[1:] - ps2[:-1])[same].min()
        assert mind >= 24, f"spacing violated: {mind}"
    return out


def prepare_in_maps(inputs):
    x = np.asarray(inputs["x"], dtype=np.float32)
    edge_index = np.asarray(inputs["edge_index"])
    w_rel1 = np.asarray(inputs["w_rel1"], dtype=np.float32)
    b_rel1 = np.asarray(inputs["b_rel1"], dtype=np.float32)
    w_root1 = np.asarray(inputs["w_root1"], dtype=np.float32)
    w_rel2 = np.asarray(inputs["w_rel2"], dtype=np.float32)
    b_rel2 = np.asarray(inputs["b_rel2"], dtype=np.float32)
    w_root2 = np.asarray(inputs["w_root2"], dtype=np.float32)

    src = edge_index[0].astype(np.int64)
    dst = edge_index[1].astype(np.int64)
    core = dst // SHARD
    dloc = dst - core * SHARD
    hh = dloc // HB
    dh = dloc - hh * HB
    gpad = (src // SHARD) * SH2 + (src % SHARD)
    rr = gpad // RSRC
    gv = (gpad % RSRC).astype(np.int16)

    key = ((core * NR + rr) * NH + hh).astype(np.int64)
    order = np.argsort(key, kind="stable")
    cnt = np.bincount(key, minlength=NC * NR * NH).reshape(NC, NR, NH)
    mx = cnt.max(axis=0)  # [NR, NH]
    NCH = [[int((mx[r, h] + CR - 1) // CR) for h in range(NH)] for r in range(NR)]
    SOFF = [[0] * NH for _ in range(NR)]
    off = 0
    for r in range(NR):
        for h in range(NH):
            SOFF[r][h] = off
            off += NCH[r][h] * CH
    GW = off
    _PROF.update(dict(NCH=NCH, SOFF=SOFF, GW=GW))

    st = np.zeros(NC * NR * NH + 1, np.int64)
    np.cumsum(cnt.reshape(-1), out=st[1:])

    aidx = np.arange(SH2, dtype=np.int16)

    xbf = np.zeros((SH2, F), bf16)
    in_maps = []
    for c_ in range(NC):
        gstream = np.zeros(GW, np.int16)
        sstream = np.full(GW, DUMP, np.int16)
        for r in range(NR):
            for h in range(NH):
                gi0 = st[(c_ * NR + r) * NH + h]
                gi1 = st[(c_ * NR + r) * NH + h + 1]
                sel = order[gi0:gi1]
                for c in range(NCH[r][h]):
                    part = sel[c * CR : (c + 1) * CR]
                    if len(part) == 0:
                        continue
                    slot_list = _space_chunk(dh[part])
                    assert len(slot_list) <= CH, (len(slot_list), CH)
                    base = SOFF[r][h] + c * CH
                    val = slot_list >= 0
                    es = part[slot_list[val]]
                    posi = base + np.flatnonzero(val)
                    gstream[posi] = gv[es]
                    sstream[posi] = dh[es].astype(np.int16)
        xsh = np.zeros((SH2, F), np.float32)
        xsh[:SHARD] = x[c_ * SHARD : (c_ + 1) * SHARD]
        b1c = np.zeros((128, 1), np.float32)
        b1c[:, 0] = b_rel1
        b2c = np.zeros((128, 1), np.float32)
        b2c[:O, 0] = b_rel2
        in_maps.append(
            {
                "xs": xsh.astype(bf16),
                "gidx": _wrap16(gstream),
                "sidx": _wrap16(sstream),
                "aidx": _wrap16(aidx),
                "wr1T": np.ascontiguousarray(w_rel1.T).astype(bf16),
                "wo1T": np.ascontiguousarray(w_root1.T).astype(bf16),
                "wr2T": np.ascontiguousarray(w_rel2.T).astype(bf16),
                "wo2T": np.ascontiguousarray(w_root2.T).astype(bf16),
                "b1c": b1c,
                "b2c": b2c,
            }
        )
    return in_maps


def get_nc():
    if "nc" not in _CACHED:
        _CACHED["nc"] = _build_program()
    return _CACHED["nc"]


def kernel(**inputs):
    from concourse.bass_utils import run_bass_kernel_spmd

    in_maps = prepare_in_maps(inputs)
    nc = get_nc()
    res = run_bass_kernel_spmd(nc, in_maps, core_ids=list(range(NC)), trace=False)
    out = np.concatenate(
        [res.results[c]["out"][:SHARD] for c in range(NC)], axis=0
    )
    return out.astype(np.float32)
